# revision 35
# baseline (speedup 1.0000x reference)
"""BNAF forward + log-det on 8 TRN2 NeuronCores (self-contained).

Sharding: data-parallel over batch (128 rows/core), params replicated.
Host does layout-only prep (transpose / gather / structural masking / constant
fills); all arithmetic on input values happens on device.

v2 design (vs baseline): main matmuls run "transposed" — activations are the
stationary operand and the weight strips are the f32r moving operand with
>=256-column segments (full PE rate), so the big W1 block-triangle is consumed
at 1 cycle/column.  Activations come out batch-partitioned in PSUM, are copied
to SBUF and PE-transposed back to feature-partition chunks for the per-feature
scale/bias tanh (ACT per-partition scale).  Weight-norm row sums (wsn) are
column-block matmuls with a constant ones stationary into a [16,128] PSUM tile
(one LDWEIGHTS total), and s = exp(dw - 0.5 ln wsn) is computed on [16,128]
tiles then PE-transposed to the [128,16] per-partition layout.  The log-det
E-path runs in bf16 feature-partition form.  All DMAs are HWDGE (f32r DRAM
params, no casting SWDGE on Pool).

Math (validated against reference):
  w = raw-lower-blocks + exp(diag-blocks);  z = (x @ w.T)*s + b
  s[r] = exp(dw[r] - 0.5*ln(wsn[r])), wsn = row sums of w^2.
  E0 = wn0_diag*(1-h0^2); E1 = blockdiag(wn1)@E0*(1-h1^2); E2 = blockdiag(wn2)@E1
  ldj = sum_d [ln(1 + e^gate * E2) - ln(1 + e^gate)] (flows 0,1); sum_d ln E2 (flow 2)
"""
import numpy as np

DIM, HID, B = 64, 32, 1024
NCORES = 8
BC = B // NCORES
DH = DIM * HID             # 2048
NK = DH // 128             # 16
LOG_2PI = float(np.log(2.0 * np.pi))
NEG = -1e30
MAIN_DT = 'float32r'       # kept for test.py compatibility

_CACHE = {}


# ---------------------------------------------------------------- host prep
def _host_prep(inputs):
    fl = {}
    r = np.arange(DH)
    blk = r // HID             # 32-block index of a DH-length axis
    c64 = np.arange(DIM)
    for f in range(3):
        W0 = np.asarray(inputs[f'W{f}_0'])
        W1 = np.asarray(inputs[f'W{f}_1'])
        W2 = np.asarray(inputs[f'W{f}_2'])

        W0T = np.ascontiguousarray(W0.T)                      # (64, 2048)
        keep = c64[:, None] < blk[None, :]
        diag = c64[:, None] == blk[None, :]
        fl[f'w0raw{f}'] = np.where(keep, W0T, 0.0).astype(np.float32)
        fl[f'w0exp{f}'] = np.where(diag, W0T, NEG).astype(np.float32)

        W1T = np.ascontiguousarray(W1.T)                      # (2048, 2048)
        # full strictly-lower 32-block triangle (incl. within-128 parts)
        keep1 = blk[:, None] < blk[None, :]
        fl[f'w1raw{f}'] = np.where(keep1, W1T, 0.0).astype(np.float32)
        # 32-diag values, NEG elsewhere, packed per 128-chunk: (128, 2048)
        d_exp = np.full((128, DH), NEG, np.float32)
        i = np.arange(128)
        dia = (i[:, None] // HID) == (i[None, :] // HID)
        for k in range(NK):
            t = W1T[128 * k:128 * k + 128, 128 * k:128 * k + 128]
            d_exp[:, 128 * k:128 * k + 128] = np.where(dia, t, NEG)
        fl[f'w1dexp{f}'] = d_exp

        W2T = np.ascontiguousarray(W2.T)                      # (2048, 64)
        keep2 = c64[None, :] > blk[:, None]
        diag2 = c64[None, :] == blk[:, None]
        w2r = np.where(keep2, W2T, 0.0).astype(np.float32)
        w2e = np.where(diag2, W2T, NEG).astype(np.float32)
        # chunk-packed: (128, NK*64)
        fl[f'w2raw{f}'] = np.ascontiguousarray(
            w2r.reshape(NK, 128, DIM).transpose(1, 0, 2).reshape(128, NK * DIM))
        fl[f'w2exp{f}'] = np.ascontiguousarray(
            w2e.reshape(NK, 128, DIM).transpose(1, 0, 2).reshape(128, NK * DIM))

        # dw in [16,128] (row m = features 128m..128m+128) for the s-glue
        fl[f'dwT0{f}'] = np.ascontiguousarray(
            np.asarray(inputs[f'dw{f}_0'])[:, 0].reshape(NK, 128)).astype(np.float32)
        fl[f'dwT1{f}'] = np.ascontiguousarray(
            np.asarray(inputs[f'dw{f}_1'])[:, 0].reshape(NK, 128)).astype(np.float32)
        fl[f'dwT2{f}'] = np.asarray(inputs[f'dw{f}_2']).reshape(1, DIM).astype(np.float32)
        # biases feature-part: [128, 16] / [64, 1]
        fl[f'b0c{f}'] = np.ascontiguousarray(
            np.asarray(inputs[f'b{f}_0']).reshape(NK, 128).T).astype(np.float32)
        fl[f'b1c{f}'] = np.ascontiguousarray(
            np.asarray(inputs[f'b{f}_1']).reshape(NK, 128).T).astype(np.float32)
        fl[f'b2c{f}'] = np.asarray(inputs[f'b{f}_2']).reshape(DIM, 1).astype(np.float32)
        # raw diag values of W0 (for c0), feature-part [128, 16]
        fl[f'g0c{f}'] = np.ascontiguousarray(
            W0[np.arange(DH), blk].reshape(NK, 128).T.astype(np.float32))

    import ml_dtypes
    fl['gatec0'] = np.full((DIM, 1), float(np.asarray(inputs['gate0'])[0]), np.float32)
    fl['gatec1'] = np.full((DIM, 1), float(np.asarray(inputs['gate1'])[0]), np.float32)
    fl['flip64'] = np.eye(DIM, dtype=np.float32)[:, ::-1].copy()
    fl['ones64'] = np.ones((DIM, 1), np.float32)
    fl['onescol'] = np.ones((128, 1), ml_dtypes.bfloat16)
    fl['ident'] = np.eye(128, dtype=np.float32)
    # selcat[:, 16m:16m+16] = column-m-only ones selector [128, 16]
    sel = np.zeros((128, NK * NK), ml_dtypes.bfloat16)
    for m in range(NK):
        sel[:, NK * m + m] = 1.0
    fl['selcat'] = sel
    return fl


# ------------------------------------------------- walrus sync-wait splitter
def _split_sync_waits(nc, max_waits=1):
    import concourse.mybir as mybir
    for func in nc.m.functions:
        for blkb in func.blocks:
            insts = list(blkb.instructions)
            out = []
            changed = False
            for inst in insts:
                si = inst.sync_info
                if si is not None and len(si.on_wait) > max_waits:
                    waits = list(si.on_wait)
                    keep, pre = waits[-max_waits:], waits[:-max_waits]
                    chunks = [pre[i:i + max_waits] for i in range(0, len(pre), max_waits)]
                    for ci, chunk in enumerate(chunks):
                        nop = mybir.InstNoOp(name=f"{inst.name}.w{ci}", ins=[], outs=[])
                        nop.engine = inst.engine
                        nop.sync_info = mybir.SyncInfo(on_wait=chunk, on_update=[])
                        out.append(nop)
                    inst.sync_info = mybir.SyncInfo(
                        on_wait=keep, on_update=list(si.on_update))
                    changed = True
                out.append(inst)
            if changed:
                try:
                    blkb.instructions = out
                except Exception:
                    while len(blkb.instructions):
                        blkb.remove_instruction(blkb.instructions[-1])
                    for i2 in out:
                        blkb.add_instruction(i2)


# ---------------------------------------------------------------- bass build
def _build(main_dt_name='float32r', repeat=1):
    import concourse.bass as bass
    import concourse.mybir as mybir
    import concourse.tile as tile

    f32 = mybir.dt.float32
    f32r = mybir.dt.float32r
    bf16 = mybir.dt.bfloat16
    AO = mybir.AluOpType
    AF = mybir.ActivationFunctionType

    nc = bass.Bass()
    P = {}

    def dram(name, shape, dt=f32r):
        P[name] = nc.declare_dram_parameter(name, list(shape), dt, isOutput=False)

    dram('xT', (DIM, BC))
    for f in range(3):
        dram(f'w0raw{f}', (DIM, DH)); dram(f'w0exp{f}', (DIM, DH))
        dram(f'w1raw{f}', (DH, DH))
        dram(f'w1dexp{f}', (128, DH))
        dram(f'w2raw{f}', (128, NK * DIM)); dram(f'w2exp{f}', (128, NK * DIM))
        dram(f'dwT0{f}', (NK, 128), f32); dram(f'dwT1{f}', (NK, 128), f32)
        dram(f'dwT2{f}', (1, DIM), f32)
        dram(f'b0c{f}', (128, NK), f32); dram(f'b1c{f}', (128, NK), f32)
        dram(f'b2c{f}', (DIM, 1), f32)
        dram(f'g0c{f}', (128, NK), f32)
    dram('gatec0', (DIM, 1), f32); dram('gatec1', (DIM, 1), f32)
    dram('flip64', (DIM, DIM)); dram('ones64', (DIM, 1))
    dram('onescol', (128, 1), bf16)
    dram('selcat', (128, NK * NK), bf16)
    dram('ident', (128, 128))
    out = nc.declare_dram_parameter('out', [1, BC], f32, isOutput=True)
    DBG = {}
    import os as _os
    if _os.environ.get('KDEBUG'):
        for nm, shp in [('d_s0', (128, NK)), ('d_s1', (128, NK)), ('d_s2', (DIM, 1)),
                        ('d_h0', (128, BC)), ('d_E0', (128, BC)), ('d_h1', (128, BC)),
                        ('d_E1', (128, BC)), ('d_E2', (DIM, BC)), ('d_z2', (DIM, BC)),
                        ('d_x1', (DIM, BC)), ('d_wsn0', (NK, 128)), ('d_wsn1', (NK, 128))]:
            DBG[nm] = nc.declare_dram_parameter(nm, list(shp), f32, isOutput=True)

    with tile.TileContext(nc) as tc:
        with tc.tile_pool(name='const', bufs=1) as cpool, \
             tc.tile_pool(name='strips', bufs=1) as stpool, \
             tc.tile_pool(name='wts', bufs=1) as wpool, \
             tc.tile_pool(name='acts', bufs=1) as apool, \
             tc.tile_pool(name='sq', bufs=3) as sqpool, \
             tc.tile_pool(name='small', bufs=1) as spool, \
             tc.tile_pool(name='pzb', bufs=2, space='PSUM') as pzb, \
             tc.tile_pool(name='ptr', bufs=2, space='PSUM') as ptr, \
             tc.tile_pool(name='pwsn', bufs=1, space='PSUM') as pwsn, \
             tc.tile_pool(name='pm', bufs=1, space='PSUM') as psm:

            def ld(pool, tag, shape, srcap, dt=f32r):
                t = pool.tile(list(shape), dt, name=tag, tag=tag)
                nc.sync.dma_start(out=t[:], in_=srcap)
                return t

            onescol = ld(cpool, 'onescol', (128, 1), P['onescol'][:], bf16)
            selcat = ld(cpool, 'selcat', (128, NK * NK), P['selcat'][:], bf16)
            ones64 = ld(cpool, 'ones64', (DIM, 1), P['ones64'][:])
            flip64 = ld(cpool, 'flip64', (DIM, DIM), P['flip64'][:])
            ident = ld(cpool, 'ident', (128, 128), P['ident'][:])
            xT0 = ld(cpool, 'xT', (DIM, BC), P['xT'][:])

            def s_glue(wsnp, dwT, ncols, tag):
                """s = exp(dwT - 0.5*ln(wsnp)) transposed to [ncols, nrows]."""
                nr = wsnp.shape[0]
                ln = spool.tile([nr, ncols], f32, name=tag + '_ln', tag=tag + '_ln')
                nc.scalar.activation(ln[:], wsnp[:], AF.Ln)
                t1 = spool.tile([nr, ncols], f32, name=tag + '_t1', tag=tag + '_t1')
                nc.vector.tensor_scalar(out=t1[:], in0=ln[:], scalar1=-0.5,
                                        scalar2=None, op0=AO.mult)
                nc.vector.tensor_tensor(t1[:], t1[:], dwT[:], AO.add)
                sT = spool.tile([nr, ncols], f32r, name=tag + '_sT', tag=tag + '_sT')
                nc.scalar.activation(sT[:], t1[:], AF.Exp)
                sp = pwsn.tile([128, NK], f32r, name=tag + '_sp', tag='wsnp')
                # moving free >= 16 keeps fp32r ISA restrictions happy; cols
                # past nr transpose to zeros we never read.
                nc.tensor.transpose(sp[:ncols, :NK], sT[:], ident[:nr, :NK])
                sp = sp[:ncols, :nr]
                s = spool.tile([ncols, nr], f32, name=tag + '_s', tag=tag + '_s')
                nc.vector.tensor_copy(s[:], sp)
                return s

            for _rep in range(repeat):
              xT = xT0
              if _rep:
                xT = ld(cpool, 'xT', (DIM, BC), P['xT'][:])
              acc = cpool.tile([DIM, BC], f32, name='acc', tag='acc')
              nc.vector.memset(acc[:], 0.0)
              E2s = [cpool.tile([DIM, BC], f32, name=f'E2_{f}', tag=f'E2_{f}')
                     for f in range(3)]
              egs = [cpool.tile([DIM, 1], f32, name=f'eg_{f}', tag=f'eg_{f}')
                     for f in range(2)]

              for f in range(3):
                    # ================= DMA =================
                    strips = [ld(stpool, f'strip{k}', (128, DH - 128 * k),
                                 P[f'w1raw{f}'][128 * k:128 * k + 128, 128 * k:])
                              for k in range(NK)]
                    w1dexpin = ld(wpool, 'w1dexpin', (128, DH), P[f'w1dexp{f}'][:])
                    w0raw = ld(wpool, 'w0raw', (DIM, DH), P[f'w0raw{f}'][:])
                    w0expin = ld(wpool, 'w0expin', (DIM, DH), P[f'w0exp{f}'][:])
                    w2rawp = ld(wpool, 'w2rawp', (128, NK * DIM), P[f'w2raw{f}'][:])
                    w2expin = ld(wpool, 'w2expin', (128, NK * DIM), P[f'w2exp{f}'][:])
                    dwT0 = ld(spool, 'dwT0', (NK, 128), P[f'dwT0{f}'][:], f32)
                    dwT1 = ld(spool, 'dwT1', (NK, 128), P[f'dwT1{f}'][:], f32)
                    dwT2 = ld(spool, 'dwT2', (1, DIM), P[f'dwT2{f}'][:], f32)
                    b0c = ld(spool, 'b0c', (128, NK), P[f'b0c{f}'][:], f32)
                    b1c = ld(spool, 'b1c', (128, NK), P[f'b1c{f}'][:], f32)
                    b2c = ld(spool, 'b2c', (DIM, 1), P[f'b2c{f}'][:], f32)
                    g0c = ld(spool, 'g0c', (128, NK), P[f'g0c{f}'][:], f32)

                    # ================= PREP =================
                    # diag exp (bf16: used as psE lhsT and added into strips)
                    w1dexp = apool.tile([128, DH], bf16, name='w1dexp', tag='w1dexp')
                    nc.scalar.activation(w1dexp[:], w1dexpin[:], AF.Exp)
                    for k in range(NK):
                        eng = (nc.vector, nc.gpsimd)[k % 2]
                        eng.tensor_tensor(strips[k][:, 0:128], strips[k][:, 0:128],
                                          w1dexp[:, 128 * k:128 * k + 128], AO.add)
                    w0exp = wpool.tile([DIM, DH], f32r, name='w0exp', tag='w0exp')
                    nc.scalar.activation(w0exp[:], w0expin[:], AF.Exp)
                    w2exp = wpool.tile([128, NK * DIM], bf16, name='w2exp', tag='w2exp')
                    nc.scalar.activation(w2exp[:], w2expin[:], AF.Exp)
                    w2comb = wpool.tile([128, NK * DIM], bf16, name='w2comb', tag='w2comb')
                    nc.vector.tensor_tensor(w2comb[:], w2rawp[:], w2exp[:], AO.add)

                    # squares (bf16)
                    sq0r = sqpool.tile([DIM, DH], bf16, name='sq0r', tag='sq0r', bufs=1)
                    nc.scalar.square(sq0r[:], w0raw[:])
                    sq0e = sqpool.tile([DIM, DH], bf16, name='sq0e', tag='sq0e', bufs=1)
                    nc.scalar.activation(sq0e[:], w0expin[:], AF.Exp, scale=2.0)
                    sqw2 = sqpool.tile([128, NK * DIM], bf16, name='sqw2', tag='sqw2',
                                       bufs=1)
                    nc.vector.tensor_tensor(sqw2[:], w2comb[:], w2comb[:], AO.mult)
                    sqstrips = []
                    for k in range(NK):
                        s = sqpool.tile([128, DH - 128 * k], bf16,
                                        name=f'sqs{k}', tag=f'sqs{k}', bufs=1)
                        eng = (nc.scalar, nc.vector, nc.gpsimd)[k % 3]
                        if eng is nc.scalar:
                            nc.scalar.square(s[:], strips[k][:])
                        else:
                            eng.tensor_tensor(s[:], strips[k][:], strips[k][:], AO.mult)
                        sqstrips.append(s)

                    # wsn col-block sums -> [16,128] psum (row m via selector col m)
                    wsn0p = pwsn.tile([NK, 128], f32, name='wsn0p', tag='wsnp')
                    for m in range(NK):
                        sel = selcat[:DIM, NK * m:NK * m + NK]
                        nc.tensor.matmul(wsn0p[:], sel,
                                         sq0r[:, 128 * m:128 * m + 128],
                                         start=(m == 0), stop=False)
                        nc.tensor.matmul(wsn0p[:], sel,
                                         sq0e[:, 128 * m:128 * m + 128],
                                         start=False, stop=(m == NK - 1))
                    s0 = s_glue(wsn0p, dwT0, 128, 's0')
                    eg0t = spool.tile([128, NK], f32, name='eg0t', tag='eg0t')
                    nc.scalar.activation(eg0t[:], g0c[:], AF.Exp)
                    c0 = spool.tile([128, NK], f32, name='c0', tag='c0')
                    nc.vector.tensor_tensor(c0[:], eg0t[:], s0[:], AO.mult)
                    c0n = spool.tile([128, NK], f32, name='c0n', tag='c0n')
                    nc.vector.tensor_scalar(out=c0n[:], in0=c0[:], scalar1=-1.0,
                                            scalar2=None, op0=AO.mult)

                    wsn1p = pwsn.tile([NK, 128], f32, name='wsn1p', tag='wsnp')
                    for m in range(NK):
                        sel = selcat[:, NK * m:NK * m + NK]
                        for k in range(m + 1):
                            nc.tensor.matmul(
                                wsn1p[:], sel,
                                sqstrips[k][:, 128 * (m - k):128 * (m - k) + 128],
                                start=(m == 0 and k == 0),
                                stop=(m == NK - 1 and k == NK - 1))
                    s1 = s_glue(wsn1p, dwT1, 128, 's1')
                    s1n = spool.tile([128, NK], f32, name='s1n', tag='s1n')
                    nc.vector.tensor_scalar(out=s1n[:], in0=s1[:], scalar1=-1.0,
                                            scalar2=None, op0=AO.mult)

                    wsn2t = pwsn.tile([NK, 128], f32, name='wsn2p', tag='wsnp')
                    wsn2p = wsn2t[0:1, 0:DIM]
                    for k in range(NK):
                        nc.tensor.matmul(wsn2p, onescol[:],
                                         sqw2[:, DIM * k:DIM * k + DIM],
                                         start=(k == 0), stop=(k == NK - 1))
                    s2 = s_glue(wsn2p, dwT2, DIM, 's2')

                    if f < 2:
                        gc = ld(spool, 'gc', (DIM, 1), P[f'gatec{f}'][:], f32)
                        nc.scalar.activation(egs[f][:], gc[:], AF.Exp)

                    # ================= CHAIN =================
                    # ---- layer 0: zb[bc, 2048] = x @ w0 (two 1024-col halves)
                    zh0 = [pzb.tile([128, DH // 2], f32, name='zb', tag='zb')
                           for _ in range(2)]
                    for g in range(4):
                        sl = zh0[g // 2][:, 512 * (g % 2):512 * (g % 2) + 512]
                        nc.tensor.matmul(sl, xT[:], w0raw[:, 512 * g:512 * g + 512],
                                         start=True, stop=False)
                        nc.tensor.matmul(sl, xT[:], w0exp[:, 512 * g:512 * g + 512],
                                         start=False, stop=True)
                    zc0 = apool.tile([128, DH], f32r, name='zc', tag='zc')
                    nc.scalar.activation(zc0[:, 0:DH // 2], zh0[0][:], AF.Copy)
                    nc.vector.tensor_copy(zc0[:, DH // 2:], zh0[1][:])

                    h0 = apool.tile([128, DH], f32r, name='h0', tag='h0')
                    E0 = apool.tile([128, DH], bf16, name='E0', tag='E0')
                    for m in range(NK):
                        cl = slice(128 * m, 128 * m + 128)
                        zt = ptr.tile([128, 128], f32r, name='ztr', tag='ztr')
                        nc.tensor.transpose(zt[:], zc0[:, cl], ident[:])
                        nc.scalar.activation(h0[:, cl], zt[:], AF.Tanh,
                                             bias=b0c[:, m:m + 1],
                                             scale=s0[:, m:m + 1])
                        hsq = spool.tile([128, BC], bf16, name='hsq', tag='hsq', bufs=2)
                        nc.vector.tensor_tensor(hsq[:], h0[:, cl], h0[:, cl], AO.mult)
                        nc.vector.tensor_scalar(out=E0[:, cl], in0=hsq[:],
                                                scalar1=c0n[:, m:m + 1],
                                                scalar2=c0[:, m:m + 1],
                                                op0=AO.mult, op1=AO.add)

                    # ---- layer 1: zb[bc, 2048] = h0 @ w1 (strips moving)
                    zh1 = [pzb.tile([128, DH // 2], f32, name='zb', tag='zb')
                           for _ in range(2)]
                    for k in range(NK):
                        lhs = h0[:, 128 * k:128 * k + 128]
                        for g in range(k // 4, 4):
                            lo = max(512 * g, 128 * k)
                            hi = 512 * (g + 1)
                            nc.tensor.matmul(
                                zh1[g // 2][:, lo - 1024 * (g // 2):hi - 1024 * (g // 2)],
                                lhs, strips[k][:, lo - 128 * k:hi - 128 * k],
                                start=(k == 0), stop=(k == min(4 * g + 3, NK - 1)))
                    zc1 = apool.tile([128, DH], f32r, name='zc', tag='zc')
                    nc.scalar.activation(zc1[:, 0:DH // 2], zh1[0][:], AF.Copy)
                    nc.vector.tensor_copy(zc1[:, DH // 2:], zh1[1][:])

                    h1 = apool.tile([128, DH], bf16, name='h1', tag='h1')
                    E1 = apool.tile([128, DH], bf16, name='E1', tag='E1')
                    for m in range(NK):
                        cl = slice(128 * m, 128 * m + 128)
                        zt = ptr.tile([128, 128], f32r, name='ztr', tag='ztr')
                        nc.tensor.transpose(zt[:], zc1[:, cl], ident[:])
                        nc.scalar.activation(h1[:, cl], zt[:], AF.Tanh,
                                             bias=b1c[:, m:m + 1], scale=s1[:, m:m + 1])
                        hsq = spool.tile([128, BC], bf16, name='hsq', tag='hsq', bufs=2)
                        nc.vector.tensor_tensor(hsq[:], h1[:, cl], h1[:, cl], AO.mult)
                        sdt = spool.tile([128, BC], bf16, name='sdt', tag='sdt', bufs=2)
                        nc.vector.tensor_scalar(out=sdt[:], in0=hsq[:],
                                                scalar1=s1n[:, m:m + 1],
                                                scalar2=s1[:, m:m + 1],
                                                op0=AO.mult, op1=AO.add)
                        pse = ptr.tile([128, BC], f32, name='pse', tag='ztr')
                        nc.tensor.matmul(pse[:], w1dexp[:, cl], E0[:, cl],
                                         start=True, stop=True)
                        nc.vector.tensor_tensor(E1[:, cl], pse[:], sdt[:], AO.mult)

                    # ---- layer 2 (feature-part, bf16); psz2/psE2 share one bank
                    pmt = psm.tile([128, BC], f32, name='pmt', tag='pm')
                    psz2 = pmt[0:DIM, :]
                    psE2 = pmt[DIM:2 * DIM, :]
                    for k in range(NK):
                        nc.tensor.matmul(psz2, w2comb[:, DIM * k:DIM * k + DIM],
                                         h1[:, 128 * k:128 * k + 128],
                                         start=(k == 0), stop=(k == NK - 1))
                    z2 = spool.tile([DIM, BC], f32, name='z2s', tag='z2s')
                    nc.scalar.activation(z2[:], psz2, AF.Identity,
                                         bias=b2c[:, 0:1], scale=s2[:, 0:1])
                    for k in range(NK):
                        nc.tensor.matmul(psE2, w2exp[:, DIM * k:DIM * k + DIM],
                                         E1[:, 128 * k:128 * k + 128],
                                         start=(k == 0), stop=(k == NK - 1))
                    nc.vector.tensor_scalar(out=E2s[f][:], in0=psE2,
                                            scalar1=s2[:, 0:1], scalar2=None,
                                            op0=AO.mult)

                    if f == 0 and DBG:
                        def _st(nm, t):
                            tmp = spool.tile(list(t.shape), f32, name='dbg' + nm,
                                             tag='dbgtmp', bufs=2)
                            nc.vector.tensor_copy(tmp[:], t[:])
                            nc.sync.dma_start(out=DBG[nm][:], in_=tmp[:])
                        _st('d_s0', s0); _st('d_s1', s1); _st('d_s2', s2)
                        _st('d_h0', h0[:, 128:256]); _st('d_E0', E0[:, 128:256])
                        _st('d_h1', h1[:, 128:256]); _st('d_E1', E1[:, 128:256])
                        _st('d_E2', E2s[0]); _st('d_z2', z2)

                    # ---- gate mix / flip or final logp term
                    if f < 2:
                        th = spool.tile([DIM, 1], f32, name='th', tag='th')
                        nc.scalar.activation(th[:], gc[:], AF.Tanh, scale=0.5)
                        sg = spool.tile([DIM, 1], f32, name='sg', tag='sg')
                        nc.vector.tensor_scalar(out=sg[:], in0=th[:], scalar1=0.5,
                                                scalar2=0.5, op0=AO.mult, op1=AO.add)
                        ta = spool.tile([DIM, BC], f32, name='ta', tag='ta')
                        nc.vector.tensor_scalar(out=ta[:], in0=z2[:], scalar1=sg[:],
                                                scalar2=None, op0=AO.mult)
                        omsg = spool.tile([DIM, 1], f32, name='omsg', tag='omsg')
                        nc.vector.tensor_scalar(out=omsg[:], in0=sg[:], scalar1=-1.0,
                                                scalar2=1.0, op0=AO.mult, op1=AO.add)
                        tb = spool.tile([DIM, BC], f32, name='tb', tag='tb')
                        nc.vector.tensor_scalar(out=tb[:], in0=xT[:], scalar1=omsg[:],
                                                scalar2=None, op0=AO.mult)
                        xmix = spool.tile([DIM, BC], f32r, name='xmix', tag='xmix')
                        nc.vector.tensor_tensor(xmix[:], ta[:], tb[:], AO.add)
                        psf = pmt[0:DIM, :]
                        nc.tensor.matmul(psf, flip64[:], xmix[:], start=True, stop=True)
                        xT = cpool.tile([DIM, BC], f32r, name=f'xT{f + 1}', tag=f'xT{f + 1}')
                        nc.scalar.activation(xT[:], psf, AF.Copy)
                        if f == 0 and DBG:
                            tmpx = spool.tile([DIM, BC], f32, name='dbgx1',
                                              tag='dbgtmp', bufs=2)
                            nc.vector.tensor_copy(tmpx[:], xT[:])
                            nc.sync.dma_start(out=DBG['d_x1'][:], in_=tmpx[:])
                    else:
                        sqx = spool.tile([DIM, BC], f32, name='sqx', tag='sqx')
                        nc.scalar.square(sqx[:], z2[:])
                        nc.vector.tensor_scalar(out=sqx[:], in0=sqx[:], scalar1=-0.5,
                                                scalar2=-0.5 * LOG_2PI,
                                                op0=AO.mult, op1=AO.add)
                        nc.vector.tensor_tensor(acc[:], acc[:], sqx[:], AO.add)

              # ---- ldj tail (ln phase)
              for f in range(2):
                    u = spool.tile([DIM, BC], f32, name='u', tag='u')
                    nc.vector.tensor_scalar(out=u[:], in0=E2s[f][:], scalar1=egs[f][:],
                                            scalar2=1.0, op0=AO.mult, op1=AO.add)
                    lf = spool.tile([DIM, BC], f32, name='lf', tag='lf')
                    nc.scalar.activation(lf[:], u[:], AF.Ln)
                    l1p = spool.tile([DIM, 1], f32, name='l1p', tag='l1p')
                    nc.vector.tensor_scalar(out=l1p[:], in0=egs[f][:], scalar1=1.0,
                                            scalar2=None, op0=AO.add)
                    nc.scalar.activation(l1p[:], l1p[:], AF.Ln)
                    nc.vector.tensor_scalar(out=lf[:], in0=lf[:], scalar1=l1p[:],
                                            scalar2=None, op0=AO.subtract)
                    nc.vector.tensor_tensor(acc[:], acc[:], lf[:], AO.add)
              lf2 = spool.tile([DIM, BC], f32, name='lf2', tag='lf2')
              nc.scalar.activation(lf2[:], E2s[2][:], AF.Ln)
              nc.vector.tensor_tensor(acc[:], acc[:], lf2[:], AO.add)

              accr = spool.tile([DIM, BC], f32r, name='accr', tag='accr')
              nc.vector.tensor_copy(accr[:], acc[:])
              pot = psm.tile([128, BC], f32, name='pout', tag='pm')
              psum_out = pot[0:1, :]
              nc.tensor.matmul(psum_out, ones64[:], accr[:], start=True, stop=True)
              outs = spool.tile([1, BC], f32, name='outs', tag='outs')
              nc.vector.tensor_copy(outs[:], psum_out)
              nc.sync.dma_start(out=out[:], in_=outs[:])

    _split_sync_waits(nc)
    return nc


# ------------------------------------------------------------------ runner
def _make_runner(nc, n_cores):
    import jax
    from jax.sharding import Mesh, PartitionSpec
    from jax.experimental.shard_map import shard_map
    import concourse.mybir as mybir
    from concourse.bass2jax import (_bass_exec_p, partition_id_tensor,
                                    install_neuronx_cc_hook)
    install_neuronx_cc_hook()
    partition_name = nc.partition_id_tensor.name if nc.partition_id_tensor else None
    in_names, out_names, out_avals = [], [], []
    for alloc in nc.m.functions[0].allocations:
        if not isinstance(alloc, mybir.MemoryLocationSet):
            continue
        name = alloc.memorylocations[0].name
        if alloc.kind == "ExternalInput":
            if name != partition_name:
                in_names.append(name)
        elif alloc.kind == "ExternalOutput":
            out_names.append(name)
            out_avals.append(jax.core.ShapedArray(
                tuple(alloc.tensor_shape), mybir.dt.np(alloc.dtype)))
    n_params = len(in_names)
    all_names = in_names + out_names + ([partition_name] if partition_name else [])

    def _body(*args):
        operands = list(args)
        if partition_name is not None:
            operands.append(partition_id_tensor())
        outs = _bass_exec_p.bind(
            *operands, out_avals=tuple(out_avals), in_names=tuple(all_names),
            out_names=tuple(out_names), lowering_input_output_aliases=(),
            sim_require_finite=False, sim_require_nnan=False, nc=nc)
        return tuple(outs)

    devices = jax.devices()[:n_cores]
    mesh = Mesh(np.asarray(devices), ("core",))
    n_outs = len(out_names)
    in_specs = (PartitionSpec("core"),) * (n_params + n_outs)
    out_specs = (PartitionSpec("core"),) * n_outs
    fn = jax.jit(shard_map(_body, mesh=mesh, in_specs=in_specs,
                           out_specs=out_specs, check_rep=False),
                 keep_unused=True)
    return fn, in_names, out_names, out_avals


def _get_runner():
    key = 'runner'
    if key not in _CACHE:
        import sys, os
        d = os.path.dirname(os.path.abspath(__file__))
        if d not in sys.path:
            sys.path.insert(0, d)
        nc = _build()
        _CACHE[key] = _make_runner(nc, NCORES)
    return _CACHE[key]


def kernel(**inputs):
    fl = _host_prep(inputs)
    x = np.asarray(inputs['x'])
    fn, in_names, out_names, out_avals = _get_runner()
    in_maps = []
    for c in range(NCORES):
        m = dict(fl)
        m['xT'] = np.ascontiguousarray(x[c * BC:(c + 1) * BC, :].T)
        in_maps.append(m)
    concat_in = [np.concatenate([np.asarray(m[name]) for m in in_maps], axis=0)
                 for name in in_names]
    concat_zeros = [np.zeros((NCORES * a.shape[0], *a.shape[1:]), a.dtype)
                    for a in out_avals]
    outs = fn(*concat_in, *concat_zeros)
    o = np.asarray(outs[0]).reshape(NCORES, BC)
    return o.reshape(B).astype(np.float32)
